# revision 1
# baseline (speedup 1.0000x reference)
"""CANModule forward kernel for 8 Trainium2 NeuronCores.

The reference computes
    new_place = relu(place_cells + ec @ W_ec + sum_i grid_i @ W_mec_i)
(the MEC grid updates are computed-then-deleted in the reference — dead
code — so W_gh*/W_gg* never need to reach the device).

Strategy: shard the HPC output dim (8192) column-wise across 8 cores
(1024 cols each).  Per core everything folds into ONE accumulated
matmul chain:
    A  = [ec (4x4096) | broadcast(concat(grids)) (4x7168)]      # [4, 11264]
    Wc = [W_ec ; W_mec0 ; W_mec1 ; W_mec2][:, shard]            # [11264, 1024]
    out_shard = relu(A @ Wc + place[shard])                     # [4, 1024]
The place bias is folded in as a K=1 matmul with a ones vector.
PE accumulates into a single [4, 1024] PSUM tile; ACT applies the relu.

Default dtype is fp16 (PE runs fp16 in one pass vs two for fp32, and the
HBM traffic halves; measured end-to-end error ~1e-4 relative). In fp16
the whole 22 MiB weight shard is SBUF-resident (11 tiles, no reuse), so
every instruction carries at most one semaphore wait. The fp32 fallback
streams weights through an 8-slot pool instead.
"""

import numpy as np

import concourse.bass as bass
import concourse.mybir as mybir
import concourse.tile as tile
from concourse.bass_utils import run_bass_kernel_spmd

N_CORES = 8
B = 4
EC = 4096
MECS = (1024, 2048, 4096)
HPC = 8192
SHARD = HPC // N_CORES          # 1024 output cols per core
K_TOTAL = EC + sum(MECS)        # 11264 contraction rows
P = 128
KC = K_TOTAL // P               # 88 K-chunks
NSPLIT = 512                    # matmul free dim = one fp32 PSUM bank
GRP = 8                         # K-chunks per DMA group (2 MiB fp16)
NGRP = KC // GRP                # 11 groups

# layout of the packed per-core constants tensor "cst" [128, CST_F]:
#   cols [0, KC*B)              : swizzled A.T  (lhsT slices, [128, 4] per K-chunk)
#   cols [KC*B, KC*B + B)       : row 0 holds B ones (lhsT of the bias matmul)
#   cols [KC*B + B, CST_F)      : row 0 holds place_cells shard [1, SHARD]
ONES_OFF = KC * B
PL_OFF = KC * B + B
CST_F = KC * B + B + SHARD

CONFIG = {"trace": False, "dtype": "f16", "strip_ceremony": False}
_CACHE = {}


def _dts():
    if CONFIG["dtype"] == "f16":
        return mybir.dt.float16, np.float16
    return mybir.dt.float32, np.float32


def _build():
    DT, _ = _dts()
    resident = CONFIG["dtype"] == "f16"
    # fp16: whole W shard lives in SBUF (22 x 8KB/partition tiles). Groups of
    # 4 K-chunks (1 MiB DMAs) keep PE idle gaps under the ~3.4us HAM
    # re-throttle window so the PE stays at 2.4 GHz.
    # fp32: stream through an 8-slot pool (one slot per DMASW sem lane so the
    # WAW dep is same-lane).
    grp = GRP
    ngrp = NGRP
    bufs = ngrp if resident else 8

    nc = bass.Bass()
    cst = nc.dram_tensor("cst", [P, CST_F], DT, kind="ExternalInput")
    # host pre-swizzles W into SBUF layout: row (g*128+p) holds chunk-major
    # (c, n) so each group DMA reads one contiguous run per partition
    w = nc.dram_tensor("w", [NGRP * P, GRP * SHARD], DT, kind="ExternalInput")
    out = nc.dram_tensor("out", [B, SHARD], mybir.dt.float32, kind="ExternalOutput")

    with tile.TileContext(nc) as tc:
        with (
            tc.tile_pool(name="const", bufs=1) as const_pool,
            tc.tile_pool(name="wload", bufs=bufs) as w_pool,
            tc.tile_pool(name="outp", bufs=1) as o_pool,
            tc.tile_pool(name="acc", bufs=1, space="PSUM") as ps_pool,
        ):
            cst_t = const_pool.tile([P, CST_F], DT)
            ps = ps_pool.tile([B, SHARD], mybir.dt.float32)

            # cst goes via the SWDGE (gpsimd) path so both HWDGE rings carry
            # nothing but the W stream — the first W matmul gates on
            # max(cst, group 0) and cst is tiny.
            nc.gpsimd.dma_start(cst_t[:], cst[:])

            # place bias first (as a K=1 rank-1 update ones[4].T @ place_shard)
            # so the first matmul depends on only the cst DMA; later matmuls
            # then carry at most one sem wait each (their W-group DMA) —
            # Matmult supports a single sync-wait command.
            for j in range(SHARD // NSPLIT):
                nc.tensor.matmul(
                    ps[:, NSPLIT * j : NSPLIT * (j + 1)],
                    cst_t[0:1, ONES_OFF : ONES_OFF + B],
                    cst_t[0:1, PL_OFF + NSPLIT * j : PL_OFF + NSPLIT * (j + 1)],
                    start=True,
                    stop=False,
                )
            w_r = w.rearrange("(g p) m -> g p m", p=P)
            for g in range(ngrp):
                wt = w_pool.tile([P, grp * SHARD], DT)
                if resident:
                    # alternate the two HWDGE rings (SP / ACT) so descriptor
                    # generation is not single-ring-limited
                    eng = nc.sync if g % 2 == 0 else nc.scalar
                    eng.dma_start(wt[:], w_r[g])
                else:
                    # SWDGE: HWDGE's direct2D pseudo-op can't carry the extra
                    # slot-reuse wait
                    nc.gpsimd.dma_start(wt[:], w_r[g])
                # runs of 2 same-bank matmuls (chunk pairs, bank-major inside
                # the pair): same wait placement as chunk-major order, but
                # half the matmuls follow a same-bank predecessor and can
                # pipeline fill-over-drain
                for c0 in range(0, grp, 2):
                    for j in range(SHARD // NSPLIT):
                        for c in (c0, c0 + 1):
                            k = g * grp + c
                            nc.tensor.matmul(
                                ps[:, NSPLIT * j : NSPLIT * (j + 1)],
                                cst_t[:, B * k : B * (k + 1)],
                                wt[:, c * SHARD + NSPLIT * j : c * SHARD + NSPLIT * (j + 1)],
                                start=False,
                                stop=(k == KC - 1),
                            )
            o_t = o_pool.tile([B, SHARD], mybir.dt.float32)
            nc.scalar.activation(o_t[:], ps[:], mybir.ActivationFunctionType.Relu)
            nc.sync.dma_start(out[:], o_t[:])

    _strip_redundant_waits(nc)
    if CONFIG["strip_ceremony"]:
        _strip_ceremony(nc)
    return nc


def _strip_ceremony(nc):
    """Remove the all-engine butterfly barriers that bracket the kernel.

    The start barrier only aligns engine boot; every data dependency in this
    kernel is carried by absolute-valued semaphore waits from a zeroed sem
    file, so engines may enter their streams unaligned. At the tail, keep the
    quiesce drain + the semaphore range-clear (needed if the NEFF is ever
    re-executed) but drop the second butterfly after it — each engine's
    stream simply ends.
    """
    blocks = nc.m.functions[0].blocks
    b0 = blocks[0]
    drop = [
        n
        for n, i in enumerate(b0.instructions)
        if type(i).__name__ in ("InstDrain", "InstEventSemaphore")
    ]
    for n in reversed(drop):
        del b0.instructions[n]

    end = blocks[-1]
    isa_idx = [
        n for n, i in enumerate(end.instructions) if type(i).__name__ == "InstISA"
    ]
    if isa_idx:
        for n in range(len(end.instructions) - 1, isa_idx[-1], -1):
            del end.instructions[n]


def _emit_group_mms(nc, cst_t, ps, wt, g, grp):
    for c in range(grp):
        k = g * grp + c
        lhsT = cst_t[:, B * k : B * (k + 1)]
        for j in range(SHARD // NSPLIT):
            nc.tensor.matmul(
                ps[:, NSPLIT * j : NSPLIT * (j + 1)],
                lhsT,
                wt[:, c, NSPLIT * j : NSPLIT * (j + 1)],
                start=False,
                stop=(k == KC - 1),
            )


def _strip_redundant_waits(nc):
    """Work around Tile's non-transitively-minimal sem assignment: the DMA /
    Matmult / Drain pseudo-ops encode a single sync wait, but Tile can emit
    more.

    1. Slot-reusing W DMAs get {PE >= x, DMASW_k >= 16m}. The DMASW_k wait
       (previous same-slot DMA fully landed) is implied by PE >= x: the
       matmuls counted by PE >= x read that slot's old contents and were
       themselves gated on DMASW_k >= 16m; PE is in-order.
    2. The end-of-kernel quiesce drain waits on every proc lane, but the
       kernel is one dependency chain ending in the output-store DMA:
       store waits ACT, ACT waits PE>=all matmuls, each matmul waited its
       W-load DMA. "Store landed" implies everything else.
    """
    insts = [i for blk in nc.m.functions[0].blocks for i in blk.instructions]
    for inst in insts:
        ty = type(inst).__name__
        si = inst.sync_info
        if si is None or len(si.on_wait) <= 1:
            continue
        if ty == "InstDMACopy":
            own_lanes = {u.ant_name for u in si.on_update}
            waits = list(si.on_wait)
            self_lane = [w for w in waits if w.ant_name in own_lanes]
            engine = [
                w
                for w in waits
                if w not in self_lane
                and w.ant_name.split("_")[0] in ("PE", "Activation", "DVE", "Pool", "SP")
            ]
            rest = [w for w in waits if w not in engine and w not in self_lane]
            if len(engine) == 1 and self_lane and not rest:
                si.on_wait = engine
                continue
        if ty in ("InstDMACopy", "InstMatmult"):
            raise RuntimeError(
                f"{inst.name} ({ty}) still has {len(si.on_wait)} waits: {si}"
            )

    store = [i for i in insts if type(i).__name__ == "InstDMACopy"][-1]
    assert store.sync_info and len(store.sync_info.on_update) == 1
    lane = store.sync_info.on_update[0].ant_name
    cum = 0
    for i in insts:
        if i.sync_info:
            cum += sum(
                u.update_value for u in i.sync_info.on_update if u.ant_name == lane
            )
    for inst in insts:
        if type(inst).__name__ != "InstDrain":
            continue
        si = inst.sync_info
        if si is None or len(si.on_wait) <= 1:
            continue
        keep = [w for w in si.on_wait if w.ant_name == lane and w.wait_value == cum]
        assert keep, f"drain {inst.name} lacks the store-lane wait (cum={cum}): {si}"
        si.on_wait = keep[:1]


def kernel(**inputs):
    _, np_dt = _dts()
    ec = np.asarray(inputs["ec_activations"], dtype=np.float32)
    place = np.asarray(inputs["place_cells"], dtype=np.float32)
    grids = [np.asarray(inputs[f"grid{i}"], dtype=np.float32) for i in range(3)]
    W_ec = np.asarray(inputs["W_ec"], dtype=np.float32)
    W_mec = [np.asarray(inputs[f"W_mec{i}"], dtype=np.float32) for i in range(3)]

    X = np.concatenate(grids, axis=1)                                   # [1, 7168]
    A = np.concatenate([ec, np.broadcast_to(X, (B, X.shape[1]))], 1)    # [4, 11264]
    # pre-swizzle A.T into the SBUF layout [p, (k m)] so the device DMA is
    # a plain contiguous copy
    aT_sw = np.ascontiguousarray(
        A.T.reshape(KC, P, B).transpose(1, 0, 2)
    ).reshape(P, KC * B)

    W_all = np.concatenate([W_ec] + W_mec, axis=0).astype(np_dt)        # [11264, 8192]

    key = "nc_" + CONFIG["dtype"]
    nc = _CACHE.get(key)
    if nc is None:
        nc = _CACHE[key] = _build()

    in_maps = []
    for c in range(N_CORES):
        cols = slice(SHARD * c, SHARD * (c + 1))
        cst = np.zeros((P, CST_F), np_dt)
        cst[:, :ONES_OFF] = aT_sw
        cst[0, ONES_OFF:PL_OFF] = 1.0
        cst[0, PL_OFF:] = place[0, cols]
        # swizzle to SBUF layout: row (g*128+p) = chunks (c, n) contiguous,
        # so each 1 MiB group DMA reads one contiguous run per partition
        w_sw = np.ascontiguousarray(
            W_all[:, cols].reshape(NGRP, GRP, P, SHARD).transpose(0, 2, 1, 3)
        ).reshape(NGRP * P, GRP * SHARD)
        in_maps.append({
            "cst": cst,
            "w": w_sw,
        })
    res = run_bass_kernel_spmd(
        nc, in_maps, core_ids=list(range(N_CORES)), trace=CONFIG["trace"]
    )
    _CACHE["last_results"] = res
    return np.concatenate([r["out"] for r in res.results], axis=1)



# revision 7
# speedup vs baseline: 1.3746x; 1.3746x over previous
"""CANModule forward kernel for 8 Trainium2 NeuronCores.

The reference computes
    new_place = relu(place_cells + ec @ W_ec + sum_i grid_i @ W_mec_i)
(the MEC grid updates are computed-then-deleted in the reference - dead
code - so W_gh*/W_gg* never need to reach the device).

Strategy: shard the HPC output dim (8192) column-wise across 8 cores
(1024 cols each).  Weights are quantized host-side to fp8 E3M4 (x32 so
the sigma~1/64..1/32 gaussians land in e3m4's normal range), halving
HBM traffic vs fp16 - the kernel is memory-bound.  The 1/32 descale is
folded into A on the host.

Per core the matmul runs W-STATIONARY: for each 128-wide tile t of the
1024 output cols and each of 88 K-chunks,
    psum_t[128, 4] += W[k-chunk, t-tile][128, 128].T-as-lhsT @ A_k[128, 4]
so W is ingested through LDWEIGHTS with FastWeightLoad (4 fp8/cycle)
instead of streaming as the moving operand (1 elem/cycle).  That keeps
PE time (~28us) under the fp8 DMA floor (~33us).  A small warmup burst
of dummy matmuls un-throttles the PE clock (HAM) before real data lands.

The place bias is applied for free in the final ACT relu via its bias
operand; out.T tiles [128, 4x8] are stored and re-assembled on host.
"""

import numpy as np
import ml_dtypes

import concourse.bass as bass
import concourse.mybir as mybir
import concourse.tile as tile
from concourse.bass_utils import run_bass_kernel_spmd

N_CORES = 8
B = 4
EC = 4096
MECS = (1024, 2048, 4096)
HPC = 8192
SHARD = HPC // N_CORES          # 1024 output cols per core
K_TOTAL = EC + sum(MECS)        # 11264 contraction rows
P = 128
KC = K_TOTAL // P               # 88 K-chunks
T = SHARD // P                  # 8 col-tiles of 128 output cols
W_SCALE = 32.0                  # e3m4 range fit; 1/32 folded into A

CONFIG = {
    "trace": False,
    "a_dtype": "f16",           # "f16" (mixed-dtype matmul) or "f8e3"
    "strip_ceremony": False,
    "warmup": 10,               # dummy matmuls to un-throttle HAM
    "tail_split": 2,            # split last col-tile's DMA into this many
}
_CACHE = {}


def _a_dts():
    if CONFIG["a_dtype"] == "f16":
        return mybir.dt.float16, np.float16
    return mybir.dt.float8e3, ml_dtypes.float8_e3m4


def _build():
    DT_A, _ = _a_dts()
    NWARM = CONFIG["warmup"]
    NSPL = CONFIG["tail_split"]
    act_scale = 1.0 if CONFIG["a_dtype"] == "f16" else 1.0 / W_SCALE

    nc = bass.Bass()
    a = nc.dram_tensor("a", [P, KC * B], DT_A, kind="ExternalInput")
    pl = nc.dram_tensor("pl", [P, T], mybir.dt.float32, kind="ExternalInput")
    w = nc.dram_tensor("w", [T * P, KC * P], mybir.dt.float8e3, kind="ExternalInput")
    out = nc.dram_tensor("out", [P, T * B], mybir.dt.float32, kind="ExternalOutput")

    with tile.TileContext(nc) as tc:
        with (
            tc.tile_pool(name="const", bufs=1) as cpool,
            tc.tile_pool(name="wload", bufs=1) as wpool,
            tc.tile_pool(name="outp", bufs=1) as opool,
            tc.tile_pool(name="acc", bufs=1, space="PSUM") as pspool,
        ):
            a_t = cpool.tile([P, KC * B], DT_A)
            pl_t = cpool.tile([P, T], mybir.dt.float32)
            warm_t = cpool.tile([P, 512], DT_A)
            o_t = opool.tile([P, T * B], mybir.dt.float32)

            # tiny constants first on each HWDGE ring so the gate matmul and
            # the relu never wait long
            nc.sync.dma_start(a_t[:], a[:])
            nc.scalar.dma_start(pl_t[:], pl[:])
            nc.vector.memset(warm_t[:], 0.0)

            ps_tiles = []
            for t in range(T - 1):
                pst = pspool.tile([P, B], mybir.dt.float32, name=f"ps{t}")
                ps_tiles.append(pst)
            # last accumulator shares its bank with the warmup scratch
            ps_last = pspool.tile([P, 512], mybir.dt.float32, name="ps_last")
            ps_tiles.append(ps_last)

            # warmup: HAM un-throttles the PE to 2.4 GHz only after ~3.4us of
            # sustained busy; burn that window on zeros while DMAs stream
            for i in range(NWARM):
                nc.tensor.matmul(
                    ps_last[:, B:512],
                    warm_t[:, 0:P],
                    warm_t[:, 0:508],
                    start=True,
                    stop=True,
                )
            # gate: first A read on the PE stream; later matmuls then carry at
            # most one sem wait (their W-group DMA)
            nc.tensor.matmul(
                ps_last[:, B : 2 * B],
                warm_t[:, 0:P],
                a_t[:, 0:B],
                start=True,
                stop=True,
            )

            w_r = w.rearrange("(t p) m -> t p m", p=P)
            w_tiles = []
            for t in range(T):
                eng = nc.sync if t % 2 == 0 else nc.scalar
                if t < T - 1:
                    wt = wpool.tile([P, KC * P], mybir.dt.float8e3, name=f"w{t}")
                    eng.dma_start(wt[:], w_r[t])
                    w_tiles.append((wt, 0, KC))
                else:
                    kc_cut = [KC * i // NSPL for i in range(NSPL + 1)]
                    for i in range(NSPL):
                        k0, k1 = kc_cut[i], kc_cut[i + 1]
                        wt = wpool.tile(
                            [P, (k1 - k0) * P], mybir.dt.float8e3, name=f"w{t}_{i}"
                        )
                        eng = nc.sync if (t + i) % 2 == 0 else nc.scalar
                        eng.dma_start(wt[:], w_r[t][:, k0 * P : k1 * P])
                        w_tiles.append((wt, k0, k1))

            ti = 0
            for t in range(T):
                ps = ps_tiles[t][:, 0:B]
                while True:
                    wt, k0, k1 = w_tiles[ti]
                    for k in range(k0, k1):
                        c = (k - k0) * P
                        nc.tensor.matmul(
                            ps,
                            wt[:, c : c + P],
                            a_t[:, B * k : B * (k + 1)],
                            start=(k == 0),
                            stop=(k == KC - 1),
                        )
                    ti += 1
                    if k1 == KC:
                        break
                nc.scalar.activation(
                    o_t[:, B * t : B * (t + 1)],
                    ps,
                    mybir.ActivationFunctionType.Relu,
                    bias=pl_t[:, t : t + 1],
                    scale=act_scale,
                )
            nc.sync.dma_start(out[:], o_t[:])

    _strip_redundant_waits(nc)
    if CONFIG["strip_ceremony"]:
        _strip_ceremony(nc)
    return nc


def _strip_redundant_waits(nc):
    """The DMA / Matmult / Drain pseudo-ops encode a single sync wait, but
    Tile can emit more.

    1. The output-store DMA gets {Activation >= 8, DMAHW_k >= 16}.  The
       Activation wait implies the DMA wait transitively: ACT is in-order and
       every ACT is gated on PE progress whose matmuls waited on that W DMA.
    2. The end-of-kernel quiesce drain waits on every proc lane, but the
       kernel is one dependency chain ending in the output-store DMA:
       "store landed" implies everything else.
    """
    insts = [i for blk in nc.m.functions[0].blocks for i in blk.instructions]
    for inst in insts:
        ty = type(inst).__name__
        si = inst.sync_info
        if si is None or len(si.on_wait) <= 1:
            continue
        if ty == "InstDMACopy":
            waits = list(si.on_wait)
            engine = [
                w
                for w in waits
                if w.ant_name.split("_")[0] in ("PE", "Activation", "DVE", "Pool", "SP")
            ]
            rest = [w for w in waits if w not in engine]
            dma_lanes = [w for w in rest if w.ant_name.startswith("DMA")]
            if len(engine) == 1 and len(dma_lanes) == len(rest):
                si.on_wait = engine
                continue
        if ty in ("InstDMACopy", "InstMatmult"):
            raise RuntimeError(
                f"{inst.name} ({ty}) still has {len(si.on_wait)} waits: {si}"
            )

    store = [i for i in insts if type(i).__name__ == "InstDMACopy"][-1]
    assert store.sync_info and len(store.sync_info.on_update) == 1
    lane = store.sync_info.on_update[0].ant_name
    cum = 0
    for i in insts:
        if i.sync_info:
            cum += sum(
                u.update_value for u in i.sync_info.on_update if u.ant_name == lane
            )
    for inst in insts:
        if type(inst).__name__ != "InstDrain":
            continue
        si = inst.sync_info
        if si is None or len(si.on_wait) <= 1:
            continue
        keep = [w for w in si.on_wait if w.ant_name == lane and w.wait_value == cum]
        assert keep, f"drain {inst.name} lacks the store-lane wait (cum={cum}): {si}"
        si.on_wait = keep[:1]


def _strip_ceremony(nc):
    """Remove the all-engine butterfly barriers that bracket the kernel.

    Every data dependency is carried by absolute-valued semaphore waits from
    a zeroed sem file, so engines may enter their streams unaligned.
    """
    blocks = nc.m.functions[0].blocks
    b0 = blocks[0]
    drop = [
        n
        for n, i in enumerate(b0.instructions)
        if type(i).__name__ in ("InstDrain", "InstEventSemaphore")
    ]
    for n in reversed(drop):
        del b0.instructions[n]

    end = blocks[-1]
    isa_idx = [
        n for n, i in enumerate(end.instructions) if type(i).__name__ == "InstISA"
    ]
    if isa_idx:
        for n in range(len(end.instructions) - 1, isa_idx[-1], -1):
            del end.instructions[n]


def kernel(**inputs):
    _, np_a = _a_dts()
    ec = np.asarray(inputs["ec_activations"], dtype=np.float32)
    place = np.asarray(inputs["place_cells"], dtype=np.float32)
    grids = [np.asarray(inputs[f"grid{i}"], dtype=np.float32) for i in range(3)]
    W_ec = np.asarray(inputs["W_ec"], dtype=np.float32)
    W_mec = [np.asarray(inputs[f"W_mec{i}"], dtype=np.float32) for i in range(3)]

    X = np.concatenate(grids, axis=1)                                   # [1, 7168]
    A = np.concatenate([ec, np.broadcast_to(X, (B, X.shape[1]))], 1)    # [4, 11264]
    if CONFIG["a_dtype"] == "f16":
        A = A / W_SCALE          # fold the W descale into A (else: ACT rescales)
    # swizzle A.T into [p, (k b)] chunk-major layout
    aT_sw = np.ascontiguousarray(
        A.T.reshape(KC, P, B).transpose(1, 0, 2)
    ).reshape(P, KC * B).astype(np_a)

    W_all = np.concatenate([W_ec] + W_mec, axis=0)                      # [11264, 8192]
    Wq = (W_all * W_SCALE).astype(ml_dtypes.float8_e3m4)

    key = "nc_" + CONFIG["a_dtype"] + str(CONFIG["strip_ceremony"])
    nc = _CACHE.get(key)
    if nc is None:
        nc = _CACHE[key] = _build()

    in_maps = []
    for c in range(N_CORES):
        cols = slice(SHARD * c, SHARD * (c + 1))
        # [t*128+p, k*128+j] = Wq[k*128+p, t*128+j]
        w_sw = np.ascontiguousarray(
            Wq[:, cols].reshape(KC, P, T, P).transpose(2, 1, 0, 3)
        ).reshape(T * P, KC * P)
        pl_sw = np.ascontiguousarray(
            place[0, cols].reshape(T, P).T
        ).astype(np.float32)
        in_maps.append({"a": aT_sw, "pl": pl_sw, "w": w_sw})
    res = run_bass_kernel_spmd(
        nc, in_maps, core_ids=list(range(N_CORES)), trace=CONFIG["trace"]
    )
    _CACHE["last_results"] = res
    outs = []
    for c in range(N_CORES):
        o = np.asarray(res.results[c]["out"])                           # [128, 8*4]
        outs.append(o.reshape(P, T, B).transpose(2, 1, 0).reshape(B, SHARD))
    return np.concatenate(outs, axis=1)


# revision 11
# speedup vs baseline: 1.5041x; 1.0942x over previous
"""CANModule forward kernel for 8 Trainium2 NeuronCores.

The reference computes
    new_place = relu(place_cells + ec @ W_ec + sum_i grid_i @ W_mec_i)
(the MEC grid updates are computed-then-deleted in the reference - dead
code - so W_gh*/W_gg* never need to reach the device).

Strategy: shard the HPC output dim (8192) column-wise across 8 cores
(1024 cols each).  Weights are quantized host-side to fp8 E3M4 (x32 so
the sigma~1/64..1/32 gaussians land in e3m4's normal range), halving
HBM traffic vs fp16 - the kernel is memory-bound.  The 1/32 descale is
folded into A on the host.

Per core the matmul runs W-STATIONARY: for each 128-wide tile t of the
1024 output cols and each of 88 K-chunks,
    psum_t[128, 4] += W[k-chunk, t-tile][128, 128].T-as-lhsT @ A_k[128, 4]
so W is ingested through LDWEIGHTS with FastWeightLoad (4 fp8/cycle)
instead of streaming as the moving operand (1 elem/cycle).  That keeps
PE time (~28us) under the fp8 DMA floor (~33us).  A small warmup burst
of dummy matmuls un-throttles the PE clock (HAM) before real data lands.

The place bias is applied for free in the final ACT relu via its bias
operand; out.T tiles [128, 4x8] are stored and re-assembled on host.
"""

import numpy as np
import ml_dtypes

import concourse.bass as bass
import concourse.mybir as mybir
import concourse.tile as tile
from concourse.bass_utils import run_bass_kernel_spmd

N_CORES = 8
B = 4
EC = 4096
MECS = (1024, 2048, 4096)
HPC = 8192
SHARD = HPC // N_CORES          # 1024 output cols per core
K_TOTAL = EC + sum(MECS)        # 11264 contraction rows
P = 128
KC = K_TOTAL // P               # 88 K-chunks
T = SHARD // P                  # 8 col-tiles of 128 output cols
W_SCALE = 32.0                  # e3m4 range fit; 1/32 folded into A

CONFIG = {
    "trace": False,
    "a_dtype": "f16",           # "f16" (mixed-dtype matmul) or "f8e3"
    "strip_ceremony": True,
    "pieces": 4,                # W DMAs per col-tile (fine-grained sems hide
                                # the slow-SDMA-engine completion skew)
}
_CACHE = {}


def _a_dts():
    if CONFIG["a_dtype"] == "f16":
        return mybir.dt.float16, np.float16
    return mybir.dt.float8e3, ml_dtypes.float8_e3m4


def _build():
    DT_A, _ = _a_dts()
    NSPL = CONFIG["pieces"]
    act_scale = 1.0 if CONFIG["a_dtype"] == "f16" else 1.0 / W_SCALE

    nc = bass.Bass()
    a = nc.dram_tensor("a", [P, KC * B], DT_A, kind="ExternalInput")
    pl = nc.dram_tensor("pl", [P, T], mybir.dt.float32, kind="ExternalInput")
    w = nc.dram_tensor("w", [T * P, KC * P], mybir.dt.float8e3, kind="ExternalInput")
    out = nc.dram_tensor("out", [P, T * B], mybir.dt.float32, kind="ExternalOutput")

    with tile.TileContext(nc) as tc:
        with (
            tc.tile_pool(name="const", bufs=1) as cpool,
            tc.tile_pool(name="wload", bufs=1) as wpool,
            tc.tile_pool(name="outp", bufs=1) as opool,
            tc.tile_pool(name="acc", bufs=1, space="PSUM") as pspool,
        ):
            a_t = cpool.tile([P, KC * B], DT_A)
            pl_t = cpool.tile([P, T], mybir.dt.float32)
            warm_t = cpool.tile([P, P], DT_A)
            o_t = opool.tile([P, T * B], mybir.dt.float32)

            # tiny constants first on each HWDGE ring so the gate matmul and
            # the relu never wait long
            nc.sync.dma_start(a_t[:], a[:])
            nc.scalar.dma_start(pl_t[:], pl[:])
            nc.vector.memset(warm_t[:], 0.0)

            ps_tiles = []
            for t in range(T):
                pst = pspool.tile([P, B + B], mybir.dt.float32, name=f"ps{t}")
                ps_tiles.append(pst)

            # gate: first A read on the PE stream; later matmuls then carry at
            # most one sem wait (their W-piece DMA)
            nc.tensor.matmul(
                ps_tiles[-1][:, B : 2 * B],
                warm_t[:, 0:P],
                a_t[:, 0:B],
                start=True,
                stop=True,
            )

            w_r = w.rearrange("(t p) m -> t p m", p=P)
            w_tiles = []
            kc_cut = [KC * i // NSPL for i in range(NSPL + 1)]
            n_dma = 0
            for t in range(T):
                for i in range(NSPL):
                    k0, k1 = kc_cut[i], kc_cut[i + 1]
                    wt = wpool.tile(
                        [P, (k1 - k0) * P], mybir.dt.float8e3, name=f"w{t}_{i}"
                    )
                    eng = nc.sync if n_dma % 2 == 0 else nc.scalar
                    eng.dma_start(wt[:], w_r[t][:, k0 * P : k1 * P])
                    w_tiles.append((wt, k0, k1))
                    n_dma += 1

            for t in range(T):
                ps = ps_tiles[t][:, 0:B]
                for wt, k0, k1 in w_tiles[t * NSPL : (t + 1) * NSPL]:
                    for k in range(k0, k1):
                        c = (k - k0) * P
                        nc.tensor.matmul(
                            ps,
                            wt[:, c : c + P],
                            a_t[:, B * k : B * (k + 1)],
                            start=(k == 0),
                            stop=(k == KC - 1),
                        )
                nc.scalar.activation(
                    o_t[:, B * t : B * (t + 1)],
                    ps,
                    mybir.ActivationFunctionType.Relu,
                    bias=pl_t[:, t : t + 1],
                    scale=act_scale,
                )
            nc.sync.dma_start(out[:], o_t[:])

    _strip_redundant_waits(nc)
    if CONFIG["strip_ceremony"]:
        _strip_ceremony(nc)
    return nc


def _strip_redundant_waits(nc):
    """The DMA / Matmult / Drain pseudo-ops encode a single sync wait, but
    Tile can emit more.

    1. The output-store DMA gets {Activation >= 8, DMAHW_k >= 16}.  The
       Activation wait implies the DMA wait transitively: ACT is in-order and
       every ACT is gated on PE progress whose matmuls waited on that W DMA.
    2. The end-of-kernel quiesce drain waits on every proc lane, but the
       kernel is one dependency chain ending in the output-store DMA:
       "store landed" implies everything else.
    """
    insts = [i for blk in nc.m.functions[0].blocks for i in blk.instructions]
    for inst in insts:
        ty = type(inst).__name__
        si = inst.sync_info
        if si is None or len(si.on_wait) <= 1:
            continue
        if ty == "InstDMACopy":
            waits = list(si.on_wait)
            engine = [
                w
                for w in waits
                if w.ant_name.split("_")[0] in ("PE", "Activation", "DVE", "Pool", "SP")
            ]
            rest = [w for w in waits if w not in engine]
            dma_lanes = [w for w in rest if w.ant_name.startswith("DMA")]
            if len(engine) == 1 and len(dma_lanes) == len(rest):
                si.on_wait = engine
                continue
        if ty in ("InstDMACopy", "InstMatmult"):
            raise RuntimeError(
                f"{inst.name} ({ty}) still has {len(si.on_wait)} waits: {si}"
            )

    store = [i for i in insts if type(i).__name__ == "InstDMACopy"][-1]
    assert store.sync_info and len(store.sync_info.on_update) == 1
    lane = store.sync_info.on_update[0].ant_name
    cum = 0
    for i in insts:
        if i.sync_info:
            cum += sum(
                u.update_value for u in i.sync_info.on_update if u.ant_name == lane
            )
    for inst in insts:
        if type(inst).__name__ != "InstDrain":
            continue
        si = inst.sync_info
        if si is None or len(si.on_wait) <= 1:
            continue
        keep = [w for w in si.on_wait if w.ant_name == lane and w.wait_value == cum]
        assert keep, f"drain {inst.name} lacks the store-lane wait (cum={cum}): {si}"
        si.on_wait = keep[:1]


def _strip_ceremony(nc):
    """Remove the all-engine butterfly barriers that bracket the kernel.

    Every data dependency is carried by absolute-valued semaphore waits from
    a zeroed sem file, so engines may enter their streams unaligned.
    """
    blocks = nc.m.functions[0].blocks
    b0 = blocks[0]
    drop = [
        n
        for n, i in enumerate(b0.instructions)
        if type(i).__name__ in ("InstDrain", "InstEventSemaphore")
    ]
    for n in reversed(drop):
        del b0.instructions[n]

    end = blocks[-1]
    isa_idx = [
        n for n, i in enumerate(end.instructions) if type(i).__name__ == "InstISA"
    ]
    if isa_idx:
        for n in range(len(end.instructions) - 1, isa_idx[-1], -1):
            del end.instructions[n]


def kernel(**inputs):
    _, np_a = _a_dts()
    ec = np.asarray(inputs["ec_activations"], dtype=np.float32)
    place = np.asarray(inputs["place_cells"], dtype=np.float32)
    grids = [np.asarray(inputs[f"grid{i}"], dtype=np.float32) for i in range(3)]
    W_ec = np.asarray(inputs["W_ec"], dtype=np.float32)
    W_mec = [np.asarray(inputs[f"W_mec{i}"], dtype=np.float32) for i in range(3)]

    X = np.concatenate(grids, axis=1)                                   # [1, 7168]
    A = np.concatenate([ec, np.broadcast_to(X, (B, X.shape[1]))], 1)    # [4, 11264]
    if CONFIG["a_dtype"] == "f16":
        A = A / W_SCALE          # fold the W descale into A (else: ACT rescales)
    # swizzle A.T into [p, (k b)] chunk-major layout
    aT_sw = np.ascontiguousarray(
        A.T.reshape(KC, P, B).transpose(1, 0, 2)
    ).reshape(P, KC * B).astype(np_a)

    W_all = np.concatenate([W_ec] + W_mec, axis=0)                      # [11264, 8192]
    Wq = (W_all * W_SCALE).astype(ml_dtypes.float8_e3m4)

    key = "nc_" + CONFIG["a_dtype"] + str(CONFIG["strip_ceremony"])
    nc = _CACHE.get(key)
    if nc is None:
        nc = _CACHE[key] = _build()

    in_maps = []
    for c in range(N_CORES):
        cols = slice(SHARD * c, SHARD * (c + 1))
        # [t*128+p, k*128+j] = Wq[k*128+p, t*128+j]
        w_sw = np.ascontiguousarray(
            Wq[:, cols].reshape(KC, P, T, P).transpose(2, 1, 0, 3)
        ).reshape(T * P, KC * P)
        pl_sw = np.ascontiguousarray(
            place[0, cols].reshape(T, P).T
        ).astype(np.float32)
        in_maps.append({"a": aT_sw, "pl": pl_sw, "w": w_sw})
    res = run_bass_kernel_spmd(
        nc, in_maps, core_ids=list(range(N_CORES)), trace=CONFIG["trace"]
    )
    _CACHE["last_results"] = res
    outs = []
    for c in range(N_CORES):
        o = np.asarray(res.results[c]["out"])                           # [128, 8*4]
        outs.append(o.reshape(P, T, B).transpose(2, 1, 0).reshape(B, SHARD))
    return np.concatenate(outs, axis=1)
